# revision 5
# baseline (speedup 1.0000x reference)
"""Trainium2 Bass kernel for fake-quant (W8A8) linear: y = fq_tok(x) @ fq_ch(w).T + b.

Full shapes: x [4, 2048, 4096] f32, w [4096, 4096] f32, b [4096] f32.
Sharding over 8 cores: 2 token groups x 4 out-channel groups.
Per core: x_sh [4096, 4096], w_sh [1024, 4096], b_sh [1024] -> y_sh [4096, 1024].

Quantized values are integers in [-127, 127], exactly representable in bf16,
so the matmul runs on the PE array in bf16 (full rate) with fp32 PSUM
accumulation - numerically equivalent to the fp32 reference einsum on the
dequantized values.  Scales are applied in the fp32 epilogue.

v2 over the 642us baseline: all 128x128 transposes move off the PE onto the
DMA xbar (`dma_start(transpose=True)`, one 1MB transpose per token tile;
out[p, kb, t] = in[t, kb*128+p] matches the 3D qxT layout directly), so the
PE does nothing but the 2048 N=512 bf16 matmuls, which stream at the 216 ns
roofline.  Startup is restructured: qwT is split into two 512-channel halves
so tile 0's cb0 matmuls only wait on w blocks 0-3, the first three x tiles
are quantized while w blocks 4-7 are still in flight, and the PE queue is
emitted cb0-first across tiles 0-2 (FIFO order would otherwise block tile 1
behind tile 0's cb1 wait on qwT half 1).

Rounding: round-half-to-even via the fp32 magic-constant trick
(v + 1.5*2^23 rounds mantissa to integer; subtract again afterwards),
matching jnp.round.  Clipping to [-128, 127] is a no-op by construction
(|x|/s <= 127 when s = amax/127) so it is skipped.

Engine split: DVE does amax + scale/reciprocal + the fp32 epilogue
(psum*sx*sw, +bias); ACT does the two rounding passes (f32 magic-add, then
magic-subtract to bf16); Sync issues all DMAs including the transposes;
PE does only matmuls.
"""

from contextlib import ExitStack

import numpy as np

import concourse.bass as bass
import concourse.mybir as mybir
import concourse.tile as tile
from concourse import bacc

P = 128
MAGIC = 12582912.0  # 1.5 * 2**23
QMAX = 127.0
EPS = 1e-8

# full problem shapes (hardcoded per harness contract)
B, S, D_IN, D_OUT = 4, 2048, 4096, 4096
TOK = B * S  # 8192
TOK_GROUPS = 2
CH_GROUPS = 4
T_SH = TOK // TOK_GROUPS  # 4096 tokens per core
O_SH = D_OUT // CH_GROUPS  # 1024 channels per core


def build_nc(T, K, O, nch=512):
    """Build the per-core Bass program: x[T,K], w[O,K], b[O] -> y[T,O]."""
    f32 = mybir.dt.float32
    bf16 = mybir.dt.bfloat16
    Copy = mybir.ActivationFunctionType.Copy
    Alu = mybir.AluOpType
    AxX = mybir.AxisListType.X

    assert T % P == 0 and K % P == 0 and O % P == 0
    TT, KB, WT = T // P, K // P, O // P
    NCH = min(nch, O)
    CB = O // NCH
    assert CB == 2 and WT == 8, "startup interleave assumes 2 halves x 4 blocks"
    WPH = WT // CB  # w blocks per qwT half

    nc = bacc.Bacc("TRN2", target_bir_lowering=False, debug=False)
    x_ap = nc.dram_tensor("x", [T, K], f32, kind="ExternalInput").ap()
    w_ap = nc.dram_tensor("w", [O, K], f32, kind="ExternalInput").ap()
    b_ap = nc.dram_tensor("b", [O], f32, kind="ExternalInput").ap()
    y_ap = nc.dram_tensor("y", [T, O], f32, kind="ExternalOutput").ap()

    with tile.TileContext(nc) as tc, ExitStack() as ctx:
        singles = ctx.enter_context(tc.tile_pool(name="singles", bufs=1))
        bigf32 = ctx.enter_context(tc.tile_pool(name="bigf32", bufs=3))
        rnd = ctx.enter_context(tc.tile_pool(name="rnd", bufs=1))
        qpool = ctx.enter_context(tc.tile_pool(name="qpool", bufs=3))
        qtpool = ctx.enter_context(tc.tile_pool(name="qtpool", bufs=3))
        stats = ctx.enter_context(tc.tile_pool(name="stats", bufs=8))
        sxpool = ctx.enter_context(tc.tile_pool(name="sxpool", bufs=5))
        opool = ctx.enter_context(tc.tile_pool(name="opool", bufs=4))
        psum_pool = ctx.enter_context(tc.tile_pool(name="psum", bufs=6, space="PSUM"))
        dram = ctx.enter_context(tc.tile_pool(name="dram", bufs=1, space="DRAM"))

        # resident: transposed quantized weights (two 512-ch halves so cb0
        # matmuls only depend on w blocks 0-3) + broadcast scale/bias rows
        qwT = [singles.tile([P, KB, NCH], bf16, name=f"qwT{h}") for h in range(CB)]
        sw_b = singles.tile([P, O], f32)
        bb_b = singles.tile([P, O], f32)
        sw_dram = dram.tile([O, 1], f32)

        def quantize(src_t, q_t, s_t):
            # per-row amax -> scale (s_t), then round src*(1/s) to q_t (bf16)
            amax = stats.tile([P, 1], f32, tag="st", name="amax")
            nc.vector.reduce_max(
                out=amax, in_=src_t, axis=AxX, apply_absolute_value=True
            )
            nc.vector.tensor_scalar(
                out=s_t, in0=amax, scalar1=1.0 / QMAX, scalar2=EPS,
                op0=Alu.mult, op1=Alu.max,
            )
            r_t = stats.tile([P, 1], f32, tag="st", name="recip")
            nc.vector.reciprocal(out=r_t, in_=s_t)
            t_t = rnd.tile([P, K], f32, tag="rnd", name="t_round")
            # round on ACT (scale is a per-partition pointer operand; the
            # Bacc event-semaphore pass legalizes its single-wait limit)
            nc.scalar.activation(
                out=t_t, in_=src_t, func=Copy, bias=MAGIC, scale=r_t[:, 0:1]
            )
            nc.scalar.activation(out=q_t, in_=t_t, func=Copy, bias=-MAGIC, scale=1.0)

        # ---- per-block / per-tile stages ----
        def w_block(wt):
            w_t = bigf32.tile([P, K], f32, tag="big", name=f"w_{wt}")
            nc.sync.dma_start(out=w_t, in_=w_ap[wt * P : (wt + 1) * P, :])
            sw = stats.tile([P, 1], f32, tag="st", name=f"sw_{wt}")
            qw = qpool.tile([P, K], bf16, tag="q", name=f"qw_{wt}")
            quantize(w_t, qw, sw)
            h, c = divmod(wt, WPH)
            # transpose on the ACT HWDGE ring: keeps the Sync ring (loads/
            # stores) free, and the qw dependency is same-engine (ACT just
            # wrote it) so the issue never blocks the ring on a wait
            nc.scalar.dma_start(
                out=qwT[h][:, :, c * P : (c + 1) * P], in_=qw, transpose=True
            )
            nc.sync.dma_start(out=sw_dram[wt * P : (wt + 1) * P, :], in_=sw)

        def load_x(tt):
            x_t = bigf32.tile([P, K], f32, tag="big", name=f"x_{tt}")
            nc.sync.dma_start(out=x_t, in_=x_ap[tt * P : (tt + 1) * P, :])
            return x_t

        def quant_x(tt, x_t):
            sx = sxpool.tile([P, 1], f32, tag="sx", name=f"sx_{tt}")
            qx = qpool.tile([P, K], bf16, tag="q", name=f"qx_{tt}")
            quantize(x_t, qx, sx)
            qxT = qtpool.tile([P, KB, P], bf16)  # qxT[f, k, t] = qx[t, k*128+f]
            nc.scalar.dma_start(out=qxT, in_=qx, transpose=True)
            return sx, qxT

        def mm_group(tt, cb, sx, qxT):
            ps = psum_pool.tile([P, NCH], f32, tag="psum", name=f"ps_{tt}_{cb}")
            for k in range(KB):
                nc.tensor.matmul(
                    ps,
                    qxT[:, k, :],
                    qwT[cb][:, k, :],
                    start=(k == 0),
                    stop=(k == KB - 1),
                )
            return ps

        def epilogue(tt, cb, sx, ps):
            o1 = opool.tile([P, NCH], f32, tag="o", name=f"o1_{tt}_{cb}")
            nc.vector.scalar_tensor_tensor(
                out=o1, in0=ps, scalar=sx[:, 0:1],
                in1=sw_b[:, cb * NCH : (cb + 1) * NCH],
                op0=Alu.mult, op1=Alu.mult,
            )
            o2 = opool.tile([P, NCH], f32, tag="o", name=f"o2_{tt}_{cb}")
            nc.vector.tensor_add(
                out=o2, in0=o1, in1=bb_b[:, cb * NCH : (cb + 1) * NCH]
            )
            nc.sync.dma_start(
                out=y_ap[tt * P : (tt + 1) * P, cb * NCH : (cb + 1) * NCH], in_=o2
            )

        # ---- startup: interleave w blocks with the first x tiles ----
        NPRE = 3  # x tiles quantized during the w phase
        x_tiles = {}
        x_tiles[0] = load_x(0)
        for wt in range(2):
            w_block(wt)
        xq = {}
        xq[0] = quant_x(0, x_tiles[0])
        for wt in range(2, WPH):
            w_block(wt)
        x_tiles[1] = load_x(1)
        w_block(WPH)
        xq[1] = quant_x(1, x_tiles[1])
        w_block(WPH + 1)
        x_tiles[2] = load_x(2)
        xq[2] = quant_x(2, x_tiles[2])
        for wt in range(WPH + 2, WT):
            w_block(wt)

        # broadcast per-channel scale & bias across partitions
        nc.sync.dma_start(
            out=sw_b,
            in_=bass.AP(tensor=sw_dram.tensor, offset=sw_dram.offset, ap=[[0, P], [1, O]]),
        )
        nc.sync.dma_start(
            out=bb_b,
            in_=bass.AP(tensor=b_ap.tensor, offset=b_ap.offset, ap=[[0, P], [1, O]]),
        )

        # ---- PE ramp: cb0 groups of tiles 0..2 first (they only need qwT
        # half 0), then their cb1 groups, then steady state ----
        pend = {}
        for tt in range(NPRE):
            pend[(tt, 0)] = mm_group(tt, 0, *xq[tt])
        for tt in range(NPRE):
            epilogue(tt, 0, xq[tt][0], pend.pop((tt, 0)))
            pend[(tt, 1)] = mm_group(tt, 1, *xq[tt])
        for tt in range(NPRE):
            epilogue(tt, 1, xq[tt][0], pend.pop((tt, 1)))

        # ---- steady state ----
        for tt in range(NPRE, TT):
            x_t = load_x(tt)
            sx, qxT = quant_x(tt, x_t)
            for cb in range(CB):
                ps = mm_group(tt, cb, sx, qxT)
                epilogue(tt, cb, sx, ps)
    nc.compile()
    return nc


_cached_nc = None


def _get_nc():
    global _cached_nc
    if _cached_nc is None:
        _cached_nc = build_nc(T_SH, D_IN, O_SH)
    return _cached_nc


def kernel(x: np.ndarray, w: np.ndarray, b: np.ndarray, _trace=False):
    from concourse.bass_utils import run_bass_kernel_spmd

    assert x.shape == (B, S, D_IN) and w.shape == (D_OUT, D_IN) and b.shape == (D_OUT,)
    x2 = np.ascontiguousarray(x.reshape(TOK, D_IN), dtype=np.float32)
    w2 = np.ascontiguousarray(w, dtype=np.float32)
    b2 = np.ascontiguousarray(b, dtype=np.float32)

    in_maps = []
    for core in range(8):
        tg, cg = divmod(core, CH_GROUPS)
        in_maps.append(
            {
                "x": np.ascontiguousarray(x2[tg * T_SH : (tg + 1) * T_SH]),
                "w": np.ascontiguousarray(w2[cg * O_SH : (cg + 1) * O_SH]),
                "b": np.ascontiguousarray(b2[cg * O_SH : (cg + 1) * O_SH]),
            }
        )

    nc = _get_nc()
    res = run_bass_kernel_spmd(nc, in_maps, core_ids=list(range(8)), trace=_trace)

    y = np.empty((TOK, D_OUT), dtype=np.float32)
    for core in range(8):
        tg, cg = divmod(core, CH_GROUPS)
        y[tg * T_SH : (tg + 1) * T_SH, cg * O_SH : (cg + 1) * O_SH] = res.results[
            core
        ]["y"]
    if _trace:
        kernel._last_results = res
    return y.reshape(B, S, D_OUT)


# revision 7
# speedup vs baseline: 1.1829x; 1.1829x over previous
"""Trainium2 Bass kernel for fake-quant (W8A8) linear: y = fq_tok(x) @ fq_ch(w).T + b.

Full shapes: x [4, 2048, 4096] f32, w [4096, 4096] f32, b [4096] f32.
Sharding over 8 cores: 2 token groups x 4 out-channel groups.
Per core: x_sh [4096, 4096], w_sh [1024, 4096], b_sh [1024] -> y_sh [4096, 1024].

Quantized values are integers in [-127, 127], exactly representable in bf16,
so the matmul runs on the PE array in bf16 (full rate) with fp32 PSUM
accumulation - numerically equivalent to the fp32 reference einsum on the
dequantized values.  Scales are applied in the fp32 epilogue.

Design (v4):
- All 128x128 transposes run on the DMA xbar (`dma_start(transpose=True)`),
  one 1MB transpose per token tile / w block, issued from the Sync HWDGE
  ring only.  The PE does nothing but the 2048 N=512 bf16 matmuls (216 ns
  roofline each).  NOTE: the transpose ucode op BLOCKS the issuing engine
  ~5us, and concurrent transposes on both HWDGE rings crash the device -
  keep every transpose on the one Sync ring.
- Steady state is an explicit software pipeline on the Sync ring:
  iteration tt issues load(tt+2), quant(tt+2) [DVE amax + ACT rounds],
  transpose(tt+1), matmul(tt), epilogue(tt).  Without this skew the
  transpose's wait for the quant chain blocks the next load on the FIFO
  ring and the tile period balloons from 13.7us to ~22us.
- Startup: all 11 loads issue before any ring-blocking transpose; x0 is
  quantized first so its transpose overlaps the w quant chain; qwT is
  split into two 512-channel halves and the PE queue runs cb0 groups of
  tiles 0-2 first so matmuls start as soon as w blocks 0-3 are resident.
  w4-7 round-pass-1 runs on DVE to unclog ACT's weight-phase queue.

Rounding: round-half-to-even via the fp32 magic-constant trick
(v + 1.5*2^23 rounds mantissa to integer; subtract again afterwards),
matching jnp.round.  Clipping to [-128, 127] is a no-op by construction
(|x|/s <= 127 when s = amax/127) so it is skipped.
"""

from contextlib import ExitStack

import numpy as np

import concourse.bass as bass
import concourse.mybir as mybir
import concourse.tile as tile
from concourse import bacc

P = 128
MAGIC = 12582912.0  # 1.5 * 2**23
QMAX = 127.0
EPS = 1e-8

# full problem shapes (hardcoded per harness contract)
B, S, D_IN, D_OUT = 4, 2048, 4096, 4096
TOK = B * S  # 8192
TOK_GROUPS = 2
CH_GROUPS = 4
T_SH = TOK // TOK_GROUPS  # 4096 tokens per core
O_SH = D_OUT // CH_GROUPS  # 1024 channels per core


def build_nc(T, K, O, nch=512):
    """Build the per-core Bass program: x[T,K], w[O,K], b[O] -> y[T,O]."""
    f32 = mybir.dt.float32
    bf16 = mybir.dt.bfloat16
    Copy = mybir.ActivationFunctionType.Copy
    Alu = mybir.AluOpType
    AxX = mybir.AxisListType.X

    assert T % P == 0 and K % P == 0 and O % P == 0
    TT, KB, WT = T // P, K // P, O // P
    NCH = min(nch, O)
    CB = O // NCH
    assert CB == 2 and WT == 8, "startup interleave assumes 2 halves x 4 blocks"
    WPH = WT // CB  # w blocks per qwT half
    NPRE = 3  # x tiles quantized+transposed during the w phase

    nc = bacc.Bacc("TRN2", target_bir_lowering=False, debug=False)
    x_ap = nc.dram_tensor("x", [T, K], f32, kind="ExternalInput").ap()
    w_ap = nc.dram_tensor("w", [O, K], f32, kind="ExternalInput").ap()
    b_ap = nc.dram_tensor("b", [O], f32, kind="ExternalInput").ap()
    y_ap = nc.dram_tensor("y", [T, O], f32, kind="ExternalOutput").ap()

    with tile.TileContext(nc) as tc, ExitStack() as ctx:
        singles = ctx.enter_context(tc.tile_pool(name="singles", bufs=1))
        bigf32 = ctx.enter_context(tc.tile_pool(name="bigf32", bufs=3))
        rnd = ctx.enter_context(tc.tile_pool(name="rnd", bufs=1))
        qpool = ctx.enter_context(tc.tile_pool(name="qpool", bufs=3))
        qtpool = ctx.enter_context(tc.tile_pool(name="qtpool", bufs=3))
        stats = ctx.enter_context(tc.tile_pool(name="stats", bufs=8))
        sxpool = ctx.enter_context(tc.tile_pool(name="sxpool", bufs=5))
        opool = ctx.enter_context(tc.tile_pool(name="opool", bufs=4))
        psum_pool = ctx.enter_context(tc.tile_pool(name="psum", bufs=6, space="PSUM"))
        dram = ctx.enter_context(tc.tile_pool(name="dram", bufs=1, space="DRAM"))

        # resident: transposed quantized weights (two 512-ch halves so cb0
        # matmuls only depend on w blocks 0-3) + broadcast scale/bias rows
        qwT = [singles.tile([P, KB, NCH], bf16, name=f"qwT{h}") for h in range(CB)]
        sw_b = singles.tile([P, O], f32)
        bb_b = singles.tile([P, O], f32)
        sw_dram = dram.tile([O, 1], f32)

        def quantize(src_t, q_t, s_t, dve_round=False):
            # per-row amax -> scale (s_t), then round src*(1/s) to q_t (bf16)
            amax = stats.tile([P, 1], f32, tag="st", name="amax")
            nc.vector.reduce_max(
                out=amax, in_=src_t, axis=AxX, apply_absolute_value=True
            )
            nc.vector.tensor_scalar(
                out=s_t, in0=amax, scalar1=1.0 / QMAX, scalar2=EPS,
                op0=Alu.mult, op1=Alu.max,
            )
            r_t = stats.tile([P, 1], f32, tag="st", name="recip")
            nc.vector.reciprocal(out=r_t, in_=s_t)
            t_t = rnd.tile([P, K], f32, tag="rnd", name="t_round")
            if dve_round:
                # pass 1 on DVE (weight-phase load balancing)
                nc.vector.tensor_scalar(
                    out=t_t, in0=src_t, scalar1=r_t[:, 0:1], scalar2=MAGIC,
                    op0=Alu.mult, op1=Alu.add,
                )
            else:
                # pass 1 on ACT (scale is a per-partition pointer operand; the
                # Bacc event-semaphore pass legalizes its single-wait limit)
                nc.scalar.activation(
                    out=t_t, in_=src_t, func=Copy, bias=MAGIC, scale=r_t[:, 0:1]
                )
            nc.scalar.activation(out=q_t, in_=t_t, func=Copy, bias=-MAGIC, scale=1.0)

        # ---- per-block / per-tile stages (loads, quant, transposes are
        # emitted in separate phases so no ring-blocking wait precedes a
        # load on the Sync FIFO) ----
        def load_w(wt):
            w_t = bigf32.tile([P, K], f32, tag="big", name=f"w_{wt}")
            nc.sync.dma_start(out=w_t, in_=w_ap[wt * P : (wt + 1) * P, :])
            return w_t

        def quant_w(wt, w_t):
            sw = stats.tile([P, 1], f32, tag="st", name=f"sw_{wt}")
            qw = qpool.tile([P, K], bf16, tag="q", name=f"qw_{wt}")
            quantize(w_t, qw, sw, dve_round=(wt >= WPH))
            return sw, qw

        def transpose_w(wt, qw):
            h, c = divmod(wt, WPH)
            nc.sync.dma_start(
                out=qwT[h][:, :, c * P : (c + 1) * P], in_=qw, transpose=True
            )

        def load_x(tt):
            x_t = bigf32.tile([P, K], f32, tag="big", name=f"x_{tt}")
            nc.sync.dma_start(out=x_t, in_=x_ap[tt * P : (tt + 1) * P, :])
            return x_t

        def quant_x(tt, x_t):
            sx = sxpool.tile([P, 1], f32, tag="sx", name=f"sx_{tt}")
            qx = qpool.tile([P, K], bf16, tag="q", name=f"qx_{tt}")
            quantize(x_t, qx, sx)
            return sx, qx

        def transpose_x(tt, qx):
            qxT = qtpool.tile([P, KB, P], bf16)  # qxT[f, k, t] = qx[t, k*128+f]
            nc.sync.dma_start(out=qxT, in_=qx, transpose=True)
            return qxT

        def mm_group(tt, cb, qxT):
            ps = psum_pool.tile([P, NCH], f32, tag="psum", name=f"ps_{tt}_{cb}")
            for k in range(KB):
                nc.tensor.matmul(
                    ps,
                    qxT[:, k, :],
                    qwT[cb][:, k, :],
                    start=(k == 0),
                    stop=(k == KB - 1),
                )
            return ps

        def epilogue(tt, cb, sx, ps):
            o1 = opool.tile([P, NCH], f32, tag="o", name=f"o1_{tt}_{cb}")
            nc.vector.scalar_tensor_tensor(
                out=o1, in0=ps, scalar=sx[:, 0:1],
                in1=sw_b[:, cb * NCH : (cb + 1) * NCH],
                op0=Alu.mult, op1=Alu.mult,
            )
            o2 = opool.tile([P, NCH], f32, tag="o", name=f"o2_{tt}_{cb}")
            nc.vector.tensor_add(
                out=o2, in0=o1, in1=bb_b[:, cb * NCH : (cb + 1) * NCH]
            )
            nc.sync.dma_start(
                out=y_ap[tt * P : (tt + 1) * P, cb * NCH : (cb + 1) * NCH], in_=o2
            )

        # ---- startup ----
        # loads first (no ring-blocking waits): x0, w0-3, x1, x2, w4-7
        x_t = {0: load_x(0)}
        w_t = {}
        for wt in range(WPH):
            w_t[wt] = load_w(wt)
        x_t[1] = load_x(1)
        x_t[2] = load_x(2)
        for wt in range(WPH, WT):
            w_t[wt] = load_w(wt)

        # quant: x0 first (its ACT chain is short -> x0T can lead the ring),
        # then w blocks with x1/x2 interleaved
        sx = {}
        qx = {}
        sw = {}
        qw = {}
        sx[0], qx[0] = quant_x(0, x_t[0])
        for wt in range(WPH):
            sw[wt], qw[wt] = quant_w(wt, w_t[wt])
        sx[1], qx[1] = quant_x(1, x_t[1])
        sx[2], qx[2] = quant_x(2, x_t[2])
        for wt in range(WPH, WT):
            sw[wt], qw[wt] = quant_w(wt, w_t[wt])

        # transposes on the Sync ring in readiness/need order
        qxT = {}
        qxT[0] = transpose_x(0, qx[0])
        for wt in range(WPH):
            transpose_w(wt, qw[wt])
        qxT[1] = transpose_x(1, qx[1])
        qxT[2] = transpose_x(2, qx[2])
        for wt in range(WPH, WT):
            transpose_w(wt, qw[wt])

        # per-channel scale stores + broadcasts (tiny, ready long before the
        # ring gets here)
        for wt in range(WT):
            nc.sync.dma_start(out=sw_dram[wt * P : (wt + 1) * P, :], in_=sw[wt])
        nc.sync.dma_start(
            out=sw_b,
            in_=bass.AP(tensor=sw_dram.tensor, offset=sw_dram.offset, ap=[[0, P], [1, O]]),
        )
        nc.sync.dma_start(
            out=bb_b,
            in_=bass.AP(tensor=b_ap.tensor, offset=b_ap.offset, ap=[[0, P], [1, O]]),
        )

        # ---- main loop: software pipeline ----
        # iteration tt: load(tt+2), quant(tt+2), transpose(tt+1), mm(tt).
        # PE ramp: cb0 groups of tiles 0-2 first (only need qwT half 0), cb1
        # groups follow once w4-7 are transposed.
        for tt in range(TT):
            if NPRE <= tt + 2 < TT:
                x_t[tt + 2] = load_x(tt + 2)
                sx[tt + 2], qx[tt + 2] = quant_x(tt + 2, x_t[tt + 2])
            if NPRE <= tt + 1 < TT:
                qxT[tt + 1] = transpose_x(tt + 1, qx.pop(tt + 1))
            if tt < NPRE:
                ps = mm_group(tt, 0, qxT[tt])
                epilogue(tt, 0, sx[tt], ps)
                if tt == NPRE - 1:
                    for t2 in range(NPRE):
                        ps = mm_group(t2, 1, qxT[t2])
                        epilogue(t2, 1, sx[t2], ps)
            else:
                for cb in range(CB):
                    ps = mm_group(tt, cb, qxT[tt])
                    epilogue(tt, cb, sx[tt], ps)
    nc.compile()
    return nc


_cached_nc = None


def _get_nc():
    global _cached_nc
    if _cached_nc is None:
        _cached_nc = build_nc(T_SH, D_IN, O_SH)
    return _cached_nc


def kernel(x: np.ndarray, w: np.ndarray, b: np.ndarray, _trace=False):
    from concourse.bass_utils import run_bass_kernel_spmd

    assert x.shape == (B, S, D_IN) and w.shape == (D_OUT, D_IN) and b.shape == (D_OUT,)
    x2 = np.ascontiguousarray(x.reshape(TOK, D_IN), dtype=np.float32)
    w2 = np.ascontiguousarray(w, dtype=np.float32)
    b2 = np.ascontiguousarray(b, dtype=np.float32)

    in_maps = []
    for core in range(8):
        tg, cg = divmod(core, CH_GROUPS)
        in_maps.append(
            {
                "x": np.ascontiguousarray(x2[tg * T_SH : (tg + 1) * T_SH]),
                "w": np.ascontiguousarray(w2[cg * O_SH : (cg + 1) * O_SH]),
                "b": np.ascontiguousarray(b2[cg * O_SH : (cg + 1) * O_SH]),
            }
        )

    nc = _get_nc()
    res = run_bass_kernel_spmd(nc, in_maps, core_ids=list(range(8)), trace=_trace)

    y = np.empty((TOK, D_OUT), dtype=np.float32)
    for core in range(8):
        tg, cg = divmod(core, CH_GROUPS)
        y[tg * T_SH : (tg + 1) * T_SH, cg * O_SH : (cg + 1) * O_SH] = res.results[
            core
        ]["y"]
    if _trace:
        kernel._last_results = res
    return y.reshape(B, S, D_OUT)
